# revision 1
# baseline (speedup 1.0000x reference)
"""Trainium2 Bass kernel for the integrate-and-fire "Integrator" layer.

Semantics (matches the JAX reference exactly):
  input  x  [4, 200, 64, 64, 8] f32, split into 2 independent time chunks of 100.
  Per neuron (b,h,w,c) and per chunk: V += x_t; if V > 2.0: spike at t, V = 0.
  Output: spike raster, permuted to [B, T, W, C, H] = [4, 200, 64, 8, 64] f32.

Strategy: pure data parallel across 8 cores. 262144 independent sequences
(4 batch x 2 chunks x 64x64x8 neurons) of length 100 -> 32768 per core,
laid out as [128 partitions, 256 free]. The time recurrence runs as a
100-step loop of 2 dependent VectorE ops per step:
    W = V + x_t                      (tensor_tensor add)
    V = (W <= theta) * W             (scalar_tensor_tensor, hard reset)
with spikes s_t = (W > theta) extracted on GpSimd (off the critical path)
into a bf16 staging buffer (0/1 exact in bf16 -> halves output DMA traffic).
"""

import numpy as np

from concourse import bacc, bass, mybir
from concourse.tile import TileContext
from concourse.bass_utils import run_bass_kernel_spmd

_THETA = 2.0
_T = 100  # chunk length (time steps per independent sequence)
_P = 128  # SBUF partitions
_F = 256  # sequences per partition per core (128*256 = 32768 per core)
_K = 10   # time steps per DMA group
_NC = 8

_B, _TT, _H, _W, _C = 4, 200, 64, 64, 8


def _build():
    nc = bacc.Bacc("TRN2", target_bir_lowering=False, debug=False)
    x = nc.declare_dram_parameter("x", [_P, _T, _F], mybir.dt.float32, isOutput=False)
    s = nc.declare_dram_parameter("s", [_P, _T, _F], mybir.dt.bfloat16, isOutput=True)
    with TileContext(nc) as tc:
        with (
            tc.tile_pool(name="xin", bufs=3) as xpool,
            tc.tile_pool(name="sout", bufs=3) as spool,
            tc.tile_pool(name="wbuf", bufs=3) as wpool,
            tc.tile_pool(name="state", bufs=1) as stpool,
        ):
            V = stpool.tile([_P, _F], mybir.dt.float32, tag="V")
            nc.vector.memset(V[:], 0.0)
            for g in range(_T // _K):
                xt = xpool.tile([_P, _K, _F], mybir.dt.float32, tag="x")
                nc.sync.dma_start(out=xt[:], in_=x[:, g * _K:(g + 1) * _K, :])
                so = spool.tile([_P, _K, _F], mybir.dt.bfloat16, tag="s")
                for k in range(_K):
                    W = wpool.tile([_P, _F], mybir.dt.float32, tag="W")
                    # add as scalar_tensor_tensor: TT's ISA struct allows fewer
                    # sync-wait slots than TSP and overflows under Tile's sems
                    nc.vector.scalar_tensor_tensor(
                        out=W[:],
                        in0=V[:],
                        scalar=0.0,
                        in1=xt[:, k, :],
                        op0=mybir.AluOpType.bypass,
                        op1=mybir.AluOpType.add,
                    )
                    nc.vector.scalar_tensor_tensor(
                        out=V[:],
                        in0=W[:],
                        scalar=_THETA,
                        in1=W[:],
                        op0=mybir.AluOpType.is_le,
                        op1=mybir.AluOpType.mult,
                    )
                    nc.gpsimd.tensor_scalar(
                        out=so[:, k, :],
                        in0=W[:],
                        scalar1=_THETA,
                        scalar2=None,
                        op0=mybir.AluOpType.is_gt,
                    )
                nc.sync.dma_start(out=s[:, g * _K:(g + 1) * _K, :], in_=so[:])
    return nc


def _shard(x):
    # [B, 200, H, W, C] -> per-core [128, 100, 256] f32, sequence-major
    xr = (
        x.reshape(_B, 2, _T, _H, _W, _C)
        .transpose(0, 1, 3, 4, 5, 2)  # [b, chunk, h, w, c, t]
        .reshape(-1, _T)              # [262144, 100]
    )
    per_core = xr.reshape(_NC, _P, _F, _T).transpose(0, 1, 3, 2)  # [8,128,100,256]
    return [np.ascontiguousarray(per_core[c]) for c in range(_NC)]


def _unshard(core_outs):
    # list of [128, 100, 256] (bf16) -> [B, T, W, C, H] f32
    sp = np.stack([np.asarray(o, dtype=np.float32) for o in core_outs])
    sp = sp.transpose(0, 1, 3, 2).reshape(_B, 2, _H, _W, _C, _T)  # [b,k,h,w,c,t]
    out = sp.transpose(0, 1, 5, 3, 4, 2).reshape(_B, _TT, _W, _C, _H)
    return np.ascontiguousarray(out)


def _run(x, trace=False):
    nc = _build()
    nc.finalize()  # run Bacc passes (multi-wait splitting etc.); PJRT path skips it
    in_maps = [{"x": xc} for xc in _shard(np.asarray(x, dtype=np.float32))]
    res = run_bass_kernel_spmd(nc, in_maps, core_ids=list(range(_NC)), trace=trace)
    out = _unshard([r["s"] for r in res.results])
    return out, res


def kernel(inputs):
    out, _ = _run(inputs, trace=False)
    return out

